# revision 1
# baseline (speedup 1.0000x reference)
"""MemoryBank kernel for 8x TRN2 NeuronCores (SPMD, batch-parallel).

Algebraic restructure (exact in real arithmetic):
    scores   = x @ (memory @ key_w).T            # fold key proj into 64-slot table
    gate_x   = x @ gate_w[:, :D].T               # ridden along as a 65th score row
    attn     = softmax(scores / 0.1)
    retrieved= attn @ (memory @ value_w.T)       # fold value proj into table
    ret_gate = attn @ (memory @ value_w.T @ gate_w[:, D:].T)
    gate     = sigmoid(gate_x + ret_gate + b)
    out      = gate * x + (1 - gate) * retrieved

This cuts matmul FLOPs 8x vs the unfused graph and makes the kernel
DMA-bound (read x once, write out once). Each core handles one batch
element, fully dim-major (x pre-transposed on host), with fp32 scores
via float32r full-rate matmuls so the sharp softmax (logit std ~226)
keeps exact fp32 argmax behavior.
"""

from contextlib import ExitStack

import numpy as np

import concourse.bass as bass
import concourse.tile as tile
from concourse import bacc
from concourse import mybir
from concourse.bass import ts
from concourse.bass_utils import run_bass_kernel_spmd
from concourse.masks import make_identity

F32 = mybir.dt.float32
F32R = mybir.dt.float32r
AX_X = mybir.AxisListType
ALU = mybir.AluOpType
ACTF = mybir.ActivationFunctionType

B = 8
L = 4096
DIM = 2048
NSLOT = 64
NCH = DIM // 128  # 16 dim chunks
TOK = 512  # tokens per tile
NT = L // TOK  # 8 tiles per core
NQ = TOK // 128  # 4 token quarters per tile


def _build(gate_b: float, use_f32r: bool = True, s_f32r: bool = False) -> bass.Bass:
    nc = bacc.Bacc("TRN2", target_bir_lowering=False, debug=False)
    FR = F32R if use_f32r else F32
    FS = F32R if s_f32r else F32

    xT = nc.dram_tensor("xT", [DIM, L], FS, kind="ExternalInput").ap()
    GT = nc.dram_tensor("GT", [DIM, NSLOT + 1], FS, kind="ExternalInput").ap()
    WvN = nc.dram_tensor("WvN", [NSLOT, DIM], F32, kind="ExternalInput").ap()
    gv = nc.dram_tensor("gv", [1, NSLOT], F32, kind="ExternalInput").ap()
    outT = nc.dram_tensor("outT", [DIM, L], F32, kind="ExternalOutput").ap()

    # dim d = c*128 + p  (chunk-major split; consistent everywhere)
    xT_v = xT.rearrange("(c p) t -> p c t", p=128)  # [128, 16, L]
    GT_v = GT.rearrange("(c p) m -> p c m", p=128)  # [128, 16, 65]
    outT_v = outT.rearrange("(c p) t -> p c t", p=128)
    WvN_v = WvN.rearrange("n (c q) -> n c q", q=128)  # [64, 16, 128]

    with tile.TileContext(nc) as tc, ExitStack() as ctx:
        consts = ctx.enter_context(tc.tile_pool(name="consts", bufs=1))
        xpool = ctx.enter_context(tc.tile_pool(name="xpool", bufs=3))
        opool = ctx.enter_context(tc.tile_pool(name="opool", bufs=2))
        work = ctx.enter_context(tc.tile_pool(name="work", bufs=3))
        small = ctx.enter_context(tc.tile_pool(name="small", bufs=4))
        psA = ctx.enter_context(tc.tile_pool(name="psA", bufs=2, space="PSUM"))
        psT = ctx.enter_context(tc.tile_pool(name="psT", bufs=2, space="PSUM"))
        psE = ctx.enter_context(tc.tile_pool(name="psE", bufs=1, space="PSUM"))
        psR = ctx.enter_context(tc.tile_pool(name="psR", bufs=2, space="PSUM"))
        psG = ctx.enter_context(tc.tile_pool(name="psG", bufs=1, space="PSUM"))

        ident = consts.tile([128, 128], F32)
        make_identity(nc, ident)
        GT_sb = consts.tile([128, NCH, NSLOT + 1], FS)
        nc.sync.dma_start(out=GT_sb, in_=GT_v)
        Wv_stage = consts.tile([NSLOT, NCH, 128], F32)
        nc.sync.dma_start(out=Wv_stage, in_=WvN_v)
        Wv_sb = consts.tile([NSLOT, NCH, 128], FR)
        nc.scalar.copy(Wv_sb, Wv_stage)
        gv_rep = consts.tile([128, NSLOT], F32)
        nc.sync.dma_start(out=gv_rep, in_=gv.to_broadcast((128, NSLOT)))
        ones_t = consts.tile([NSLOT + 1, 128], FR)
        # memset can't target f32r; ACT Copy with scale=0 bias=1 writes ones
        nc.scalar.activation(
            ones_t, ident[0 : NSLOT + 1, :], func=ACTF.Copy, bias=1.0, scale=0.0
        )

        def phase_A_dma(t):
            """prefetch x^T tile (2 tiles ahead of use)."""
            x_sb = xpool.tile([128, NCH, TOK], FS, tag="x_sb")
            nc.sync.dma_start(out=x_sb, in_=xT_v[:, :, ts(t, TOK)])
            return {"x_sb": x_sb}

        def phase_A_mm(t, st):
            """S^T matmuls -> ACT copy -> token-major transposes."""
            x_sb = st["x_sb"]
            S_ps = psA.tile([128, TOK], F32, tag="A")
            for c in range(NCH):
                nc.tensor.matmul(
                    S_ps[0 : NSLOT + 1, :],
                    GT_sb[:, c, :],
                    x_sb[:, c, :],
                    start=(c == 0),
                    stop=(c == NCH - 1),
                )
            S_sb = work.tile([NSLOT + 1, TOK], F32, tag="S_sb")
            nc.scalar.copy(S_sb, S_ps[0 : NSLOT + 1, :])
            Stok = psT.tile([128, NQ, NSLOT + 1], F32, tag="T")
            for q in range(NQ):
                nc.tensor.transpose(
                    Stok[:, q, :],
                    S_sb[:, ts(q, 128)],
                    ident[0 : NSLOT + 1, 0 : NSLOT + 1],
                )
            st["Stok"] = Stok

        def phase_B(t, st):
            """Per-token-quarter softmax stats (DVE/ACT only)."""
            Stok = st["Stok"]
            Etok = small.tile([128, NQ, NSLOT], F32, tag="Etok")
            Ec = small.tile([128, NQ, NSLOT + 1], F32, tag="Ec")
            # ACT functions are clustered (all Exp, later all Sigmoid): every
            # function switch costs a ~1.3us ACT_TABLE_LOAD on this runtime.
            mbs, rs, gls, gts = [], [], [], []
            for q in range(NQ):
                St = Stok[:, q, :]  # cols 0..63 = scores, col 64 = gate_x
                mx = small.tile([128, 1], F32, tag="mx")
                nc.vector.tensor_reduce(mx, St[:, 0:NSLOT], axis=AX_X.X, op=ALU.max)
                mb = small.tile([128, 1], F32, tag="mb")
                nc.vector.tensor_scalar_mul(mb, mx, -10.0)
                mbs.append(mb)
            for q in range(NQ):
                nc.scalar.activation(
                    Etok[:, q, :],
                    Stok[:, q, 0:NSLOT],
                    func=ACTF.Exp,
                    bias=mbs[q],
                    scale=10.0,
                )
            for q in range(NQ):
                St = Stok[:, q, :]
                sums = small.tile([128, 1], F32, tag="sums")
                nc.vector.tensor_reduce(sums, Etok[:, q, :], axis=AX_X.X, op=ALU.add)
                scr = small.tile([128, NSLOT], F32, tag="scr")
                nc.vector.tensor_mul(scr, Etok[:, q, :], gv_rep)
                gvd = small.tile([128, 1], F32, tag="gvd")
                nc.vector.tensor_reduce(gvd, scr, axis=AX_X.X, op=ALU.add)
                r = small.tile([128, 1], F32, tag="r")
                nc.vector.reciprocal(r, sums)
                t1 = small.tile([128, 1], F32, tag="t1")
                nc.vector.tensor_mul(t1, gvd, r)
                gl = small.tile([128, 1], F32, tag="gl")
                nc.vector.tensor_add(gl, t1, St[:, NSLOT : NSLOT + 1])
                rs.append(r)
                gls.append(gl)
            for q in range(NQ):
                g_t = small.tile([128, 1], F32, tag="g_t")
                nc.scalar.activation(
                    g_t, gls[q], func=ACTF.Sigmoid, bias=gate_b, scale=1.0
                )
                gts.append(g_t)
            for q in range(NQ):
                r, g_t = rs[q], gts[q]
                nc.vector.tensor_copy(Ec[:, q, NSLOT : NSLOT + 1], g_t)
                t2 = small.tile([128, 1], F32, tag="t2")
                nc.vector.tensor_mul(t2, g_t, r)
                cp = small.tile([128, 1], F32, tag="cp")
                nc.vector.tensor_sub(cp, t2, r)
                nc.vector.tensor_scalar_mul(Ec[:, q, 0:NSLOT], Etok[:, q, :], cp)
            st["Ec"] = Ec

        def phase_C(t, st, s_next=None):
            """Back to slot-major, g broadcast, retrieval + combine, out DMA.

            s_next=(t2, st2): interleave tile t2's S-matmul accumulation
            between this tile's R matmuls so PE never idles while DVE
            paces the combine (PSUM groups are per-bank, so this is legal).
            """
            x_sb, Ec = st["x_sb"], st["Ec"]
            ET = psE.tile([NSLOT + 1, NQ, 128], F32, tag="E")
            for q in range(NQ):
                nc.tensor.transpose(ET[:, q, :], Ec[:, q, :], ident)
            E_sb = work.tile([NSLOT + 1, NQ, 128], FR, tag="E_sb")
            nc.scalar.copy(E_sb, ET)
            E_flat = E_sb.rearrange("p a b -> p (a b)")  # [65, 512]
            g_bc = psG.tile([128, TOK], F32, tag="G")
            nc.tensor.matmul(
                g_bc,
                ones_t[NSLOT : NSLOT + 1, :],
                E_flat[NSLOT : NSLOT + 1, :],
                start=True,
                stop=True,
            )
            g_sb = work.tile([128, TOK], F32, tag="g_sb")
            nc.scalar.copy(g_sb, g_bc)
            if s_next is not None:
                t2, st2 = s_next
                S_ps2 = psA.tile([128, TOK], F32, tag="A")
            for a in range(4):
                o4 = opool.tile([128, 4, TOK], F32, tag="o4")
                for cc in range(4):
                    c = 4 * a + cc
                    R_ps = psR.tile([128, TOK], F32, tag="R")
                    nc.tensor.matmul(
                        R_ps,
                        Wv_sb[:, c, :],
                        E_flat[0:NSLOT, :],
                        start=True,
                        stop=True,
                    )
                    if s_next is not None:
                        nc.tensor.matmul(
                            S_ps2[0 : NSLOT + 1, :],
                            GT_sb[:, c, :],
                            st2["x_sb"][:, c, :],
                            start=(c == 0),
                            stop=(c == NCH - 1),
                            skip_group_check=True,
                        )
                    xg = work.tile([128, TOK], F32, tag="xg")
                    xin = x_sb[:, c, :].bitcast(F32) if s_f32r else x_sb[:, c, :]
                    if c < 10:
                        nc.gpsimd.tensor_mul(xg, xin, g_sb)
                    else:
                        nc.vector.tensor_mul(xg, xin, g_sb)
                    nc.vector.tensor_add(o4[:, cc, :], xg, R_ps)
                nc.sync.dma_start(
                    out=outT_v[:, 4 * a : 4 * a + 4, ts(t, TOK)],
                    in_=o4,
                )
            if s_next is not None:
                S_sb2 = work.tile([NSLOT + 1, TOK], F32, tag="S_sb")
                nc.scalar.copy(S_sb2, S_ps2[0 : NSLOT + 1, :])
                Stok2 = psT.tile([128, NQ, NSLOT + 1], F32, tag="T")
                for q in range(NQ):
                    nc.tensor.transpose(
                        Stok2[:, q, :],
                        S_sb2[:, ts(q, 128)],
                        ident[0 : NSLOT + 1, 0 : NSLOT + 1],
                    )
                st2["Stok"] = Stok2

        # software pipeline: x-DMA 2 tiles ahead; tile t+2's S matmuls are
        # interleaved into tile t's R phase (phase_C) so PE never idles.
        states = {}
        states[0] = phase_A_dma(0)
        states[1] = phase_A_dma(1)
        phase_A_mm(0, states[0])
        phase_A_mm(1, states[1])
        phase_B(0, states[0])
        for t in range(NT):
            if t + 2 < NT:
                states[t + 2] = phase_A_dma(t + 2)
            if t + 1 < NT:
                # stats for t+1 go on DVE *before* tile t's combine so the
                # PE never stalls on Ec at the next tile boundary
                phase_B(t + 1, states[t + 1])
            phase_C(
                t,
                states[t],
                s_next=(t + 2, states[t + 2]) if t + 2 < NT else None,
            )
            del states[t]

    nc.compile()
    return nc


def _fold_weights(memory, key_w, value_w, gate_w):
    mem = memory.astype(np.float64)
    Ws = (mem @ key_w.astype(np.float64)).astype(np.float32)  # [64, 2048]
    Wv = (mem @ value_w.astype(np.float64).T).astype(np.float32)  # [64, 2048]
    gx = np.asarray(gate_w[0, :DIM], dtype=np.float32)
    gvv = (Wv.astype(np.float64) @ gate_w[0, DIM:].astype(np.float64)).astype(
        np.float32
    )
    G = np.concatenate([Ws, gx[None, :]], axis=0)  # [65, 2048]; gate row last
    GT = np.ascontiguousarray(G.T)  # [2048, 65]
    WvN = np.ascontiguousarray(-Wv)  # [64, 2048]
    return GT, WvN, gvv.reshape(1, NSLOT)


def kernel(x, memory, key_w, value_w, gate_w, gate_b, _trace=False, _tmpdir=None, _use_f32r=True, _s_f32r=False):
    x = np.asarray(x, dtype=np.float32)
    GT, WvN, gvv = _fold_weights(
        np.asarray(memory, np.float32),
        np.asarray(key_w, np.float32),
        np.asarray(value_w, np.float32),
        np.asarray(gate_w, np.float32),
    )
    nc = _build(float(np.asarray(gate_b).reshape(-1)[0]), use_f32r=_use_f32r, s_f32r=_s_f32r)
    in_maps = [
        {"xT": np.ascontiguousarray(x[b].T), "GT": GT, "WvN": WvN, "gv": gvv}
        for b in range(B)
    ]
    res = run_bass_kernel_spmd(
        nc, in_maps, list(range(B)), trace=_trace, tmpdir=_tmpdir
    )
    out = np.stack([res.results[b]["outT"].T for b in range(B)], axis=0)
    if _trace:
        return out.astype(np.float32), res
    return out.astype(np.float32)



# revision 14
# speedup vs baseline: 1.4052x; 1.4052x over previous
"""MemoryBank kernel for 8x TRN2 NeuronCores (SPMD, batch-parallel).

Algebraic restructure (exact in real arithmetic):
    S        = x @ [Ws; gx_w].T          # [L, 65]; Ws = memory @ key_w (folded)
    E        = exp(10*(S[:, :64] - m))   # m = per-token max
    sum      = E.1,  r = 1/sum
    gl       = (E @ gv) * r + S[:, 64] + b      # gv = Wv @ gate_w[D:]
    u        = exp(-gl); g = 1/(1+u)            # sigmoid without Sigmoid LUT
    out      = g * (x + (E * u * r) @ Wv)       # since (1-g)/g == u exactly

The final form lets the PE accumulate x (identity matmul, f32r) and the
retrieval (bf16) into the SAME PSUM bank, so the elementwise combine is a
single DVE multiply by a broadcast gate row. Scores run f32r (full rate),
retrieval runs bf16, output is written bf16 (host upcasts). Each core
handles one batch element; x is pre-tiled on the host so every DMA is a
single fully-contiguous 2-4 MiB transfer.
"""

from contextlib import ExitStack

import numpy as np

F16_NP = np.float16

import concourse.bass as bass
import concourse.tile as tile
from concourse import bacc
from concourse import mybir
from concourse.bass import ts
from concourse.bass_utils import run_bass_kernel_spmd
from concourse.masks import make_identity

F32 = mybir.dt.float32
F32R = mybir.dt.float32r
F16 = mybir.dt.float16
AX_X = mybir.AxisListType
ALU = mybir.AluOpType
ACTF = mybir.ActivationFunctionType

B = 8
L = 4096
DIM = 2048
NSLOT = 64
NCH = DIM // 128  # 16 dim chunks
TOK = 512  # tokens per tile
NT = L // TOK  # 8 tiles per core
NQ = TOK // 128  # 4 token quarters per tile


def _build(gate_b: float, interleave: bool = True) -> bass.Bass:
    nc = bacc.Bacc("TRN2", target_bir_lowering=False, debug=False)

    xhd = nc.dram_tensor("xhd", [NT * 128, NCH * TOK], F16, kind="ExternalInput").ap()
    xld = nc.dram_tensor("xld", [NT * 128, NCH * TOK], F16, kind="ExternalInput").ap()
    Ghd = nc.dram_tensor(
        "Ghd", [128, NCH * (NSLOT + 1)], F16, kind="ExternalInput"
    ).ap()
    Gld = nc.dram_tensor(
        "Gld", [128, NCH * (NSLOT + 1)], F16, kind="ExternalInput"
    ).ap()
    Wvd = nc.dram_tensor("Wvd", [NSLOT, NCH * 128], F16, kind="ExternalInput").ap()
    gvd = nc.dram_tensor("gvd", [1, NQ * NSLOT], F32, kind="ExternalInput").ap()
    outb = nc.dram_tensor(
        "outb", [NT * 128, NCH * TOK], F16, kind="ExternalOutput"
    ).ap()

    # per-tile views; dim d = c*128 + p everywhere
    xh_v = xhd.rearrange("(t p) (c k) -> t p c k", p=128, k=TOK)
    xl_v = xld.rearrange("(t p) (c k) -> t p c k", p=128, k=TOK)
    outb_v = outb.rearrange("(t p) (c k) -> t p c k", p=128, k=TOK)
    Gh_v = Ghd.rearrange("p (c n) -> p c n", n=NSLOT + 1)
    Gl_v = Gld.rearrange("p (c n) -> p c n", n=NSLOT + 1)
    Wv_v = Wvd.rearrange("n (c j) -> n c j", j=128)

    with tile.TileContext(nc) as tc, ExitStack() as ctx:
        consts = ctx.enter_context(tc.tile_pool(name="consts", bufs=1))
        xpool = ctx.enter_context(tc.tile_pool(name="xpool", bufs=3))
        opool = ctx.enter_context(tc.tile_pool(name="opool", bufs=2))
        work = ctx.enter_context(tc.tile_pool(name="work", bufs=2))
        small = ctx.enter_context(tc.tile_pool(name="small", bufs=2))
        psS = ctx.enter_context(tc.tile_pool(name="psS", bufs=1, space="PSUM"))
        psT = ctx.enter_context(tc.tile_pool(name="psT", bufs=1, space="PSUM"))
        psE = ctx.enter_context(tc.tile_pool(name="psE", bufs=1, space="PSUM"))
        psG = ctx.enter_context(tc.tile_pool(name="psG", bufs=1, space="PSUM"))
        psP = ctx.enter_context(tc.tile_pool(name="psP", bufs=2, space="PSUM"))

        ident = consts.tile([128, 128], F32)
        make_identity(nc, ident)
        ident_16 = consts.tile([128, 128], F16)
        nc.vector.tensor_copy(ident_16, ident)
        Gh_sb = consts.tile([128, NCH, NSLOT + 1], F16)
        nc.sync.dma_start(out=Gh_sb, in_=Gh_v)
        Gl_sb = consts.tile([128, NCH, NSLOT + 1], F16)
        nc.sync.dma_start(out=Gl_sb, in_=Gl_v)
        Wv_sb = consts.tile([NSLOT, NCH, 128], F16)
        nc.sync.dma_start(out=Wv_sb, in_=Wv_v)
        gv4_flat = consts.tile([128, NQ * NSLOT], F32)
        nc.sync.dma_start(out=gv4_flat, in_=gvd.to_broadcast((128, NQ * NSLOT)))
        gv4 = gv4_flat.rearrange("p (a b) -> p a b", b=NSLOT)
        ones_16 = consts.tile([NSLOT + 1, 128], F16)
        nc.vector.memset(ones_16, 1.0)

        def ph_dma(t):
            xh_sb = xpool.tile([128, NCH, TOK], F16, tag="xh")
            nc.sync.dma_start(out=xh_sb, in_=xh_v[t])
            xl_sb = xpool.tile([128, NCH, TOK], F16, tag="xl")
            nc.sync.dma_start(out=xl_sb, in_=xl_v[t])
            return {"xh": xh_sb, "xl": xl_sb}

        def s_passes(st, c):
            # fp16-pair split: G@x = Gh@xh + Gh@xl + Gl@xh (+ O(2^-24))
            return (
                (Gh_sb[:, c, :], st["xh"][:, c, :]),
                (Gh_sb[:, c, :], st["xl"][:, c, :]),
                (Gl_sb[:, c, :], st["xh"][:, c, :]),
            )

        def ph_S(t, st):
            """S^T = [Ws; gxw] @ x_tile via 3 fp16 passes per chunk."""
            S_ps = psS.tile([128, TOK], F32, tag="S")
            for c in range(NCH):
                for i, (lhsT, rhs) in enumerate(s_passes(st, c)):
                    nc.tensor.matmul(
                        S_ps[0 : NSLOT + 1, :],
                        lhsT,
                        rhs,
                        start=(c == 0 and i == 0),
                        stop=(c == NCH - 1 and i == 2),
                    )
            st["S_ps"] = S_ps

        def ph_Stok(t, st):
            S_ps = st.pop("S_ps")
            S_sb = work.tile([NSLOT + 1, TOK], F32, tag="S_sb")
            nc.scalar.copy(S_sb, S_ps[0 : NSLOT + 1, :])
            Stok = psT.tile([128, NQ, NSLOT + 1], F32, tag="T")
            for q in range(NQ):
                nc.tensor.transpose(
                    Stok[:, q, :],
                    S_sb[:, ts(q, 128)],
                    ident[0 : NSLOT + 1, 0 : NSLOT + 1],
                )
            st["Stok"] = Stok

        def ph_stats(t, st):
            """Token-major softmax stats; produces Ec (bf16, col 64 = g)."""
            Stok = st.pop("Stok")
            mx4 = small.tile([128, NQ], F32, tag="mx4")
            nc.vector.tensor_reduce(mx4, Stok[:, :, 0:NSLOT], axis=AX_X.X, op=ALU.max)
            mb4 = small.tile([128, NQ], F32, tag="mb4")
            nc.vector.tensor_scalar_mul(mb4, mx4, -10.0)
            Etok = work.tile([128, NQ, NSLOT], F32, tag="Etok")
            s4 = small.tile([128, NQ], F32, tag="s4")
            for q in range(NQ):
                nc.scalar.activation(
                    Etok[:, q, :],
                    Stok[:, q, 0:NSLOT],
                    func=ACTF.Exp,
                    bias=mb4[:, q : q + 1],
                    scale=10.0,
                    accum_out=s4[:, q : q + 1],
                )
            gvd4 = small.tile([128, NQ], F32, tag="gvd4")
            scr = work.tile([128, NQ, NSLOT], F32, tag="scr")
            nc.vector.tensor_mul(scr, Etok, gv4)
            nc.vector.tensor_reduce(gvd4, scr, axis=AX_X.X, op=ALU.add)
            r4 = small.tile([128, NQ], F32, tag="r4")
            nc.vector.reciprocal(r4, s4)
            t4 = small.tile([128, NQ], F32, tag="t4")
            nc.vector.tensor_mul(t4, gvd4, r4)
            gl4 = small.tile([128, NQ], F32, tag="gl4")
            gx_row = Stok[:, :, NSLOT : NSLOT + 1].rearrange("p a b -> p (a b)")
            nc.vector.tensor_add(gl4, t4, gx_row)
            u4 = small.tile([128, NQ], F32, tag="u4")
            nc.scalar.activation(
                u4, gl4, func=ACTF.Exp, bias=-gate_b, scale=-1.0
            )
            den4 = small.tile([128, NQ], F32, tag="den4")
            nc.vector.tensor_scalar_add(den4, u4, 1.0)
            g4 = small.tile([128, NQ], F32, tag="g4")
            nc.vector.reciprocal(g4, den4)
            cp4 = small.tile([128, NQ], F32, tag="cp4")
            nc.vector.tensor_mul(cp4, u4, r4)
            Ec = work.tile([128, NQ, NSLOT + 1], F16, tag="Ec")
            for q in range(NQ):
                nc.vector.tensor_scalar_mul(
                    Ec[:, q, 0:NSLOT], Etok[:, q, :], cp4[:, q : q + 1]
                )
            g_col = Ec[:, :, NSLOT : NSLOT + 1].rearrange("p a b -> p (a b)")
            nc.vector.tensor_copy(g_col, g4)
            st["Ec"] = Ec

        def ph_E(t, st):
            """Ec back to slot-major (bf16): E_sb[n, tok], row 64 = g."""
            Ec = st.pop("Ec")
            ET = psE.tile([128, NQ, 128], F16, tag="ET")
            for q in range(NQ):
                nc.tensor.transpose(ET[0 : NSLOT + 1, q, :], Ec[:, q, :], ident_16)
            E_sb = work.tile([NSLOT + 1, NQ, 128], F16, tag="E_sb")
            nc.scalar.copy(E_sb, ET[0 : NSLOT + 1, :, :])
            st["E_sb"] = E_sb

        def ph_combine(t, st, s_next=None):
            """P = x + E''@Wv accumulated on PE per PSUM bank-pair; then
            out = g * P on DVE; one contiguous bf16 DMA per tile."""
            x = st["xh"]
            E_flat = st.pop("E_sb").rearrange("p a b -> p (a b)")  # [65, 512]
            g_ps = psG.tile([128, TOK], F32, tag="G")
            nc.tensor.matmul(
                g_ps,
                ones_16[NSLOT : NSLOT + 1, :],
                E_flat[NSLOT : NSLOT + 1, :],
                start=True,
                stop=True,
            )
            g2 = work.tile([128, 2, TOK], F32, tag="g2")
            nc.scalar.copy(g2[:, 0, :], g_ps)
            nc.scalar.copy(g2[:, 1, :], g_ps)
            out_sb = opool.tile([128, NCH, TOK], F16, tag="o")
            for j in range(NCH // 2):
                P = psP.tile([128, 2, TOK], F32, tag="P")
                for k in range(2):
                    c = 2 * j + k
                    nc.tensor.matmul(
                        P[:, k, :],
                        ident_16,
                        x[:, c, :],
                        start=True,
                        stop=False,
                    )
                    nc.tensor.matmul(
                        P[:, k, :],
                        Wv_sb[:, c, :],
                        E_flat[0:NSLOT, :],
                        start=False,
                        stop=True,
                    )
                if s_next is not None:
                    # interleave 6 of next tile's S matmuls per pair
                    t2, st2 = s_next
                    if j == 0:
                        st2["_Sps"] = psS.tile(
                            [128, TOK], F32, tag="S", name="S_ps_il"
                        )
                    for k in range(2):
                        c = 2 * j + k
                        for i, (lhsT, rhs) in enumerate(s_passes(st2, c)):
                            nc.tensor.matmul(
                                st2["_Sps"][0 : NSLOT + 1, :],
                                lhsT,
                                rhs,
                                start=(c == 0 and i == 0),
                                stop=(c == NCH - 1 and i == 2),
                                skip_group_check=True,
                            )
                nc.vector.tensor_mul(out_sb[:, 2 * j : 2 * j + 2, :], P, g2)
            nc.sync.dma_start(out=outb_v[t], in_=out_sb)
            if s_next is not None:
                t2, st2 = s_next
                st2["S_ps"] = st2.pop("_Sps")
                ph_Stok(t2, st2)

        # Software pipeline: per iteration t the PE stream is
        #   ET(t) -> g_bc(t) -> [P pairs(t) + S(t+1) interleaved] -> Stok(t+1)
        # DVE: combine TTs(t) -> stats(t+1);  x DMA runs 2 tiles ahead.
        states = {0: ph_dma(0), 1: ph_dma(1)}
        ph_S(0, states[0])
        ph_Stok(0, states[0])
        ph_stats(0, states[0])
        for t in range(NT):
            if t + 2 < NT:
                states[t + 2] = ph_dma(t + 2)
            ph_E(t, states[t])
            ph_combine(
                t,
                states[t],
                s_next=(t + 1, states[t + 1])
                if (interleave and t + 1 < NT)
                else None,
            )
            if t + 1 < NT:
                if not interleave:
                    ph_S(t + 1, states[t + 1])
                    ph_Stok(t + 1, states[t + 1])
                ph_stats(t + 1, states[t + 1])
            del states[t]

    nc.compile()
    return nc


def _fold_weights(memory, key_w, value_w, gate_w):
    mem = np.asarray(memory, np.float64)
    # query = x @ key_w.T ; scores = query @ memory.T = x @ (memory @ key_w).T
    Ws = (mem @ np.asarray(key_w, np.float64)).astype(np.float32)  # [64, 2048]
    Wv = (mem @ np.asarray(value_w, np.float64).T).astype(np.float32)  # [64, 2048]
    gx = np.asarray(gate_w[0, :DIM], dtype=np.float32)
    gv = (Wv.astype(np.float64) @ np.asarray(gate_w[0, DIM:], np.float64)).astype(
        np.float32
    )
    G = np.concatenate([Ws, gx[None, :]], axis=0)  # [65, 2048], gate row last
    # host layout [128, 16, 65]: GT[p, c, n] = G[n, c*128+p]; fp16 pair
    GT = np.ascontiguousarray(
        G.T.reshape(NCH, 128, NSLOT + 1).transpose(1, 0, 2)
    ).reshape(128, NCH * (NSLOT + 1))
    Gh = GT.astype(np.float16)
    Gl = (GT - Gh.astype(np.float32)).astype(np.float16)
    WvT = np.ascontiguousarray(Wv.reshape(NSLOT, NCH * 128))  # [64, 2048]
    gv4 = np.tile(gv, NQ).reshape(1, NQ * NSLOT)
    return Gh, Gl, WvT, gv4


def _tile_x(xb):
    # [L, D] -> [NT*128, NCH*TOK]: blob[t, p, c, k] = x[t*TOK+k, c*128+p]
    return np.ascontiguousarray(
        xb.reshape(NT, TOK, NCH, 128).transpose(0, 3, 2, 1)
    ).reshape(NT * 128, NCH * TOK)


def _untile_out(blob):
    # [NT*128, NCH*TOK] -> [L, D]
    return (
        blob.reshape(NT, 128, NCH, TOK)
        .transpose(0, 3, 2, 1)
        .reshape(L, DIM)
        .astype(np.float32)
    )


def kernel(
    x,
    memory,
    key_w,
    value_w,
    gate_w,
    gate_b,
    _trace=False,
    _tmpdir=None,
):
    x = np.asarray(x, dtype=np.float32)
    Gh, Gl, WvT, gv4 = _fold_weights(
        np.asarray(memory, np.float32),
        np.asarray(key_w, np.float32),
        np.asarray(value_w, np.float32),
        np.asarray(gate_w, np.float32),
    )
    Wv_16 = WvT.astype(F16_NP)
    nc = _build(float(np.asarray(gate_b).reshape(-1)[0]))
    in_maps = []
    for b in range(B):
        xt = _tile_x(x[b])
        xh = xt.astype(np.float16)
        xl = (xt - xh.astype(np.float32)).astype(np.float16)
        in_maps.append(
            {"xhd": xh, "xld": xl, "Ghd": Gh, "Gld": Gl, "Wvd": Wv_16, "gvd": gv4}
        )
    res = run_bass_kernel_spmd(
        nc, in_maps, list(range(B)), trace=_trace, tmpdir=_tmpdir
    )
    out = np.stack(
        [_untile_out(np.asarray(res.results[b]["outb"])) for b in range(B)], axis=0
    )
    if _trace:
        return out.astype(np.float32), res
    return out.astype(np.float32)


# revision 15
# speedup vs baseline: 1.4255x; 1.0145x over previous
"""MemoryBank kernel for 8x TRN2 NeuronCores (SPMD, batch-parallel).

Algebraic restructure (exact in real arithmetic):
    S        = x @ [Ws; gx_w].T          # [L, 65]; Ws = memory @ key_w (folded)
    E        = exp(10*(S[:, :64] - m))   # m = per-token max
    sum      = E.1,  r = 1/sum
    gl       = (E @ gv) * r + S[:, 64] + b      # gv = Wv @ gate_w[D:]
    u        = exp(-gl); g = 1/(1+u)            # sigmoid without Sigmoid LUT
    out      = g * (x + (E * u * r) @ Wv)       # since (1-g)/g == u exactly

The final form lets the PE accumulate x (identity matmul, f32r) and the
retrieval (bf16) into the SAME PSUM bank, so the elementwise combine is a
single DVE multiply by a broadcast gate row. Scores run f32r (full rate),
retrieval runs bf16, output is written bf16 (host upcasts). Each core
handles one batch element; x is pre-tiled on the host so every DMA is a
single fully-contiguous 2-4 MiB transfer.
"""

from contextlib import ExitStack

import numpy as np

F16_NP = np.float16

import concourse.bass as bass
import concourse.tile as tile
from concourse import bacc
from concourse import mybir
from concourse.bass import ts
from concourse.bass_utils import run_bass_kernel_spmd
from concourse.masks import make_identity

F32 = mybir.dt.float32
F32R = mybir.dt.float32r
F16 = mybir.dt.float16
AX_X = mybir.AxisListType
ALU = mybir.AluOpType
ACTF = mybir.ActivationFunctionType

B = 8
L = 4096
DIM = 2048
NSLOT = 64
NCH = DIM // 128  # 16 dim chunks
TOK = 512  # tokens per tile
NT = L // TOK  # 8 tiles per core
NQ = TOK // 128  # 4 token quarters per tile


def _build(gate_b: float, interleave: bool = True) -> bass.Bass:
    nc = bacc.Bacc("TRN2", target_bir_lowering=False, debug=False)

    xhd = nc.dram_tensor("xhd", [NT * 128, NCH * TOK], F16, kind="ExternalInput").ap()
    xld = nc.dram_tensor("xld", [NT * 128, NCH * TOK], F16, kind="ExternalInput").ap()
    Ghd = nc.dram_tensor(
        "Ghd", [128, NCH * 128], F16, kind="ExternalInput"
    ).ap()
    Gld = nc.dram_tensor(
        "Gld", [128, NCH * 128], F16, kind="ExternalInput"
    ).ap()
    Wvd = nc.dram_tensor("Wvd", [NSLOT, NCH * 128], F16, kind="ExternalInput").ap()
    gvd = nc.dram_tensor("gvd", [1, NQ * NSLOT], F32, kind="ExternalInput").ap()
    outb = nc.dram_tensor(
        "outb", [NT * 128, NCH * TOK], F16, kind="ExternalOutput"
    ).ap()

    # per-tile views; dim d = c*128 + p everywhere
    xh_v = xhd.rearrange("(t p) (c k) -> t p c k", p=128, k=TOK)
    xl_v = xld.rearrange("(t p) (c k) -> t p c k", p=128, k=TOK)
    outb_v = outb.rearrange("(t p) (c k) -> t p c k", p=128, k=TOK)
    Gh_v = Ghd.rearrange("p (c n) -> p c n", n=128)
    Gl_v = Gld.rearrange("p (c n) -> p c n", n=128)
    Wv_v = Wvd.rearrange("n (c j) -> n c j", j=128)

    with tile.TileContext(nc) as tc, ExitStack() as ctx:
        consts = ctx.enter_context(tc.tile_pool(name="consts", bufs=1))
        xpool = ctx.enter_context(tc.tile_pool(name="xpool", bufs=3))
        opool = ctx.enter_context(tc.tile_pool(name="opool", bufs=2))
        work = ctx.enter_context(tc.tile_pool(name="work", bufs=2))
        small = ctx.enter_context(tc.tile_pool(name="small", bufs=2))
        psS = ctx.enter_context(tc.tile_pool(name="psS", bufs=1, space="PSUM"))
        psT = ctx.enter_context(tc.tile_pool(name="psT", bufs=1, space="PSUM"))
        psE = ctx.enter_context(tc.tile_pool(name="psE", bufs=1, space="PSUM"))
        psG = ctx.enter_context(tc.tile_pool(name="psG", bufs=1, space="PSUM"))
        psP = ctx.enter_context(tc.tile_pool(name="psP", bufs=2, space="PSUM"))

        ident = consts.tile([128, 128], F32)
        make_identity(nc, ident)
        ident_16 = consts.tile([128, 128], F16)
        nc.vector.tensor_copy(ident_16, ident)
        Gh_sb = consts.tile([128, NCH, 128], F16)
        nc.sync.dma_start(out=Gh_sb, in_=Gh_v)
        Gl_sb = consts.tile([128, NCH, 128], F16)
        nc.sync.dma_start(out=Gl_sb, in_=Gl_v)
        Wv_sb = consts.tile([NSLOT, NCH, 128], F16)
        nc.sync.dma_start(out=Wv_sb, in_=Wv_v)
        gv4_flat = consts.tile([128, NQ * NSLOT], F32)
        nc.sync.dma_start(out=gv4_flat, in_=gvd.to_broadcast((128, NQ * NSLOT)))
        gv4 = gv4_flat.rearrange("p (a b) -> p a b", b=NSLOT)
        ones_16 = consts.tile([NSLOT + 1, 128], F16)
        nc.vector.memset(ones_16, 1.0)

        def ph_dma(t):
            xh_sb = xpool.tile([128, NCH, TOK], F16, tag="xh")
            nc.sync.dma_start(out=xh_sb, in_=xh_v[t])
            xl_sb = xpool.tile([128, NCH, TOK], F16, tag="xl")
            nc.sync.dma_start(out=xl_sb, in_=xl_v[t])
            return {"xh": xh_sb, "xl": xl_sb}

        def s_seq(st):
            # fp16-pair split: G@x = Gh@xh + Gl@xh + Gh@xl (+ O(2^-24)),
            # pass-major so the first 32 matmuls only need xh
            seq = []
            for G_c, xkey in ((Gh_sb, "xh"), (Gl_sb, "xh"), (Gh_sb, "xl")):
                for c in range(NCH):
                    seq.append((G_c[:, c, :], st[xkey][:, c, :]))
            return seq

        def ph_S(t, st):
            """S^T = [Ws; gxw] @ x_tile via 3 fp16 passes per chunk."""
            S_ps = psS.tile([128, TOK], F32, tag="S")
            seq = s_seq(st)
            for i, (lhsT, rhs) in enumerate(seq):
                nc.tensor.matmul(
                    S_ps,
                    lhsT,
                    rhs,
                    start=(i == 0),
                    stop=(i == len(seq) - 1),
                )
            st["S_ps"] = S_ps

        def ph_Stok(t, st):
            S_ps = st.pop("S_ps")
            S_sb = work.tile([NSLOT + 1, TOK], F32, tag="S_sb")
            nc.scalar.copy(S_sb, S_ps[0 : NSLOT + 1, :])
            Stok = psT.tile([128, NQ, NSLOT + 1], F32, tag="T")
            for q in range(NQ):
                nc.tensor.transpose(
                    Stok[:, q, :],
                    S_sb[:, ts(q, 128)],
                    ident[0 : NSLOT + 1, 0 : NSLOT + 1],
                )
            st["Stok"] = Stok

        def ph_stats(t, st):
            """Token-major softmax stats; produces Ec (bf16, col 64 = g)."""
            Stok = st.pop("Stok")
            mx4 = small.tile([128, NQ], F32, tag="mx4")
            nc.vector.tensor_reduce(mx4, Stok[:, :, 0:NSLOT], axis=AX_X.X, op=ALU.max)
            mb4 = small.tile([128, NQ], F32, tag="mb4")
            nc.vector.tensor_scalar_mul(mb4, mx4, -10.0)
            Etok = work.tile([128, NQ, NSLOT], F32, tag="Etok")
            s4 = small.tile([128, NQ], F32, tag="s4")
            for q in range(NQ):
                nc.scalar.activation(
                    Etok[:, q, :],
                    Stok[:, q, 0:NSLOT],
                    func=ACTF.Exp,
                    bias=mb4[:, q : q + 1],
                    scale=10.0,
                    accum_out=s4[:, q : q + 1],
                )
            gvd4 = small.tile([128, NQ], F32, tag="gvd4")
            scr = work.tile([128, NQ, NSLOT], F32, tag="scr")
            nc.vector.tensor_mul(scr, Etok, gv4)
            nc.vector.tensor_reduce(gvd4, scr, axis=AX_X.X, op=ALU.add)
            r4 = small.tile([128, NQ], F32, tag="r4")
            nc.vector.reciprocal(r4, s4)
            t4 = small.tile([128, NQ], F32, tag="t4")
            nc.vector.tensor_mul(t4, gvd4, r4)
            gl4 = small.tile([128, NQ], F32, tag="gl4")
            gx_row = Stok[:, :, NSLOT : NSLOT + 1].rearrange("p a b -> p (a b)")
            nc.vector.tensor_add(gl4, t4, gx_row)
            u4 = small.tile([128, NQ], F32, tag="u4")
            nc.scalar.activation(
                u4, gl4, func=ACTF.Exp, bias=-gate_b, scale=-1.0
            )
            den4 = small.tile([128, NQ], F32, tag="den4")
            nc.vector.tensor_scalar_add(den4, u4, 1.0)
            g4 = small.tile([128, NQ], F32, tag="g4")
            nc.vector.reciprocal(g4, den4)
            cp4 = small.tile([128, NQ], F32, tag="cp4")
            nc.vector.tensor_mul(cp4, u4, r4)
            Ec = work.tile([128, NQ, NSLOT + 1], F16, tag="Ec")
            for q in range(NQ):
                nc.vector.tensor_scalar_mul(
                    Ec[:, q, 0:NSLOT], Etok[:, q, :], cp4[:, q : q + 1]
                )
            g_col = Ec[:, :, NSLOT : NSLOT + 1].rearrange("p a b -> p (a b)")
            nc.vector.tensor_copy(g_col, g4)
            st["Ec"] = Ec

        def ph_E(t, st):
            """Ec back to slot-major (bf16): E_sb[n, tok], row 64 = g."""
            Ec = st.pop("Ec")
            ET = psE.tile([128, NQ, 128], F16, tag="ET")
            for q in range(NQ):
                nc.tensor.transpose(ET[0 : NSLOT + 1, q, :], Ec[:, q, :], ident_16)
            E_sb = work.tile([NSLOT + 1, NQ, 128], F16, tag="E_sb")
            nc.scalar.copy(E_sb, ET[0 : NSLOT + 1, :, :])
            st["E_sb"] = E_sb

        def ph_combine(t, st, s_next=None):
            """P = x + E''@Wv accumulated on PE per PSUM bank-pair; then
            out = g * P on DVE; one contiguous bf16 DMA per tile."""
            x = st["xh"]
            E_flat = st.pop("E_sb").rearrange("p a b -> p (a b)")  # [65, 512]
            g_ps = psG.tile([128, TOK], F32, tag="G")
            nc.tensor.matmul(
                g_ps,
                ones_16[NSLOT : NSLOT + 1, :],
                E_flat[NSLOT : NSLOT + 1, :],
                start=True,
                stop=True,
            )
            g2 = work.tile([128, 2, TOK], F32, tag="g2")
            nc.scalar.copy(g2[:, 0, :], g_ps)
            nc.scalar.copy(g2[:, 1, :], g_ps)
            out_sb = opool.tile([128, NCH, TOK], F16, tag="o")
            for j in range(NCH // 2):
                P = psP.tile([128, 2, TOK], F32, tag="P")
                for k in range(2):
                    c = 2 * j + k
                    nc.tensor.matmul(
                        P[:, k, :],
                        ident_16,
                        x[:, c, :],
                        start=True,
                        stop=False,
                    )
                    nc.tensor.matmul(
                        P[:, k, :],
                        Wv_sb[:, c, :],
                        E_flat[0:NSLOT, :],
                        start=False,
                        stop=True,
                    )
                if s_next is not None:
                    # interleave 6 of next tile's S matmuls per pair
                    t2, st2 = s_next
                    if j == 0:
                        st2["_Sps"] = psS.tile(
                            [128, TOK], F32, tag="S", name="S_ps_il"
                        )
                        st2["_seq"] = s_seq(st2)
                    seq = st2["_seq"]
                    for i in range(6 * j, 6 * j + 6):
                        lhsT, rhs = seq[i]
                        nc.tensor.matmul(
                            st2["_Sps"],
                            lhsT,
                            rhs,
                            start=(i == 0),
                            stop=(i == len(seq) - 1),
                            skip_group_check=True,
                        )
                nc.vector.tensor_mul(out_sb[:, 2 * j : 2 * j + 2, :], P, g2)
            nc.sync.dma_start(out=outb_v[t], in_=out_sb)
            if s_next is not None:
                t2, st2 = s_next
                st2.pop("_seq")
                st2["S_ps"] = st2.pop("_Sps")
                ph_Stok(t2, st2)

        # Software pipeline: per iteration t the PE stream is
        #   ET(t) -> g_bc(t) -> [P pairs(t) + S(t+1) interleaved] -> Stok(t+1)
        # DVE: combine TTs(t) -> stats(t+1);  x DMA runs 2 tiles ahead.
        states = {0: ph_dma(0), 1: ph_dma(1)}
        ph_S(0, states[0])
        ph_Stok(0, states[0])
        ph_stats(0, states[0])
        for t in range(NT):
            if t + 2 < NT:
                states[t + 2] = ph_dma(t + 2)
            ph_E(t, states[t])
            ph_combine(
                t,
                states[t],
                s_next=(t + 1, states[t + 1])
                if (interleave and t + 1 < NT)
                else None,
            )
            if t + 1 < NT:
                if not interleave:
                    ph_S(t + 1, states[t + 1])
                    ph_Stok(t + 1, states[t + 1])
                ph_stats(t + 1, states[t + 1])
            del states[t]

    nc.compile()
    return nc


def _fold_weights(memory, key_w, value_w, gate_w):
    mem = np.asarray(memory, np.float64)
    # query = x @ key_w.T ; scores = query @ memory.T = x @ (memory @ key_w).T
    Ws = (mem @ np.asarray(key_w, np.float64)).astype(np.float32)  # [64, 2048]
    Wv = (mem @ np.asarray(value_w, np.float64).T).astype(np.float32)  # [64, 2048]
    gx = np.asarray(gate_w[0, :DIM], dtype=np.float32)
    gv = (Wv.astype(np.float64) @ np.asarray(gate_w[0, DIM:], np.float64)).astype(
        np.float32
    )
    G = np.concatenate(
        [Ws, gx[None, :], np.zeros((128 - NSLOT - 1, DIM), np.float32)], axis=0
    )  # [128, 2048]: 64 slots, gate row, zero padding (FWL wants 128 cols)
    GT = np.ascontiguousarray(
        G.T.reshape(NCH, 128, 128).transpose(1, 0, 2)
    ).reshape(128, NCH * 128)
    Gh = GT.astype(np.float16)
    Gl = (GT - Gh.astype(np.float32)).astype(np.float16)
    WvT = np.ascontiguousarray(Wv.reshape(NSLOT, NCH * 128))  # [64, 2048]
    gv4 = np.tile(gv, NQ).reshape(1, NQ * NSLOT)
    return Gh, Gl, WvT, gv4


def _tile_x(xb):
    # [L, D] -> [NT*128, NCH*TOK]: blob[t, p, c, k] = x[t*TOK+k, c*128+p]
    return np.ascontiguousarray(
        xb.reshape(NT, TOK, NCH, 128).transpose(0, 3, 2, 1)
    ).reshape(NT * 128, NCH * TOK)


def _untile_out(blob):
    # [NT*128, NCH*TOK] -> [L, D]
    return (
        blob.reshape(NT, 128, NCH, TOK)
        .transpose(0, 3, 2, 1)
        .reshape(L, DIM)
        .astype(np.float32)
    )


def kernel(
    x,
    memory,
    key_w,
    value_w,
    gate_w,
    gate_b,
    _trace=False,
    _tmpdir=None,
):
    x = np.asarray(x, dtype=np.float32)
    Gh, Gl, WvT, gv4 = _fold_weights(
        np.asarray(memory, np.float32),
        np.asarray(key_w, np.float32),
        np.asarray(value_w, np.float32),
        np.asarray(gate_w, np.float32),
    )
    Wv_16 = WvT.astype(F16_NP)
    nc = _build(float(np.asarray(gate_b).reshape(-1)[0]))
    in_maps = []
    for b in range(B):
        xt = _tile_x(x[b])
        xh = xt.astype(np.float16)
        xl = (xt - xh.astype(np.float32)).astype(np.float16)
        in_maps.append(
            {"xhd": xh, "xld": xl, "Ghd": Gh, "Gld": Gl, "Wvd": Wv_16, "gvd": gv4}
        )
    res = run_bass_kernel_spmd(
        nc, in_maps, list(range(B)), trace=_trace, tmpdir=_tmpdir
    )
    out = np.stack(
        [_untile_out(np.asarray(res.results[b]["outb"])) for b in range(B)], axis=0
    )
    if _trace:
        return out.astype(np.float32), res
    return out.astype(np.float32)


# revision 18
# speedup vs baseline: 1.6175x; 1.1347x over previous
"""MemoryBank kernel for 8x TRN2 NeuronCores (SPMD, batch-parallel).

Algebraic restructure (exact in real arithmetic):
    S        = x @ [Ws; gx_w].T          # [L, 65]; Ws = memory @ key_w (folded)
    E        = exp(10*(S[:, :64] - m))   # m = per-token max
    sum      = E.1,  r = 1/sum
    gl       = (E @ gv) * r + S[:, 64] + b      # gv = Wv @ gate_w[D:]
    u        = exp(-gl); g = 1/(1+u)            # sigmoid without Sigmoid LUT
    out      = g * (x + (E * u * r) @ Wv)       # since (1-g)/g == u exactly

The final form lets the PE accumulate x (identity matmul, f32r) and the
retrieval (bf16) into the SAME PSUM bank, so the elementwise combine is a
single DVE multiply by a broadcast gate row. Scores run f32r (full rate),
retrieval runs bf16, output is written bf16 (host upcasts). Each core
handles one batch element; x is pre-tiled on the host so every DMA is a
single fully-contiguous 2-4 MiB transfer.
"""

from contextlib import ExitStack

import numpy as np

F16_NP = np.float16

import concourse.bass as bass
import concourse.tile as tile
from concourse import bacc
from concourse import mybir
from concourse.bass import ts
from concourse.bass_utils import run_bass_kernel_spmd
from concourse.masks import make_identity

F32 = mybir.dt.float32
F32R = mybir.dt.float32r
F16 = mybir.dt.float16
AX_X = mybir.AxisListType
ALU = mybir.AluOpType
ACTF = mybir.ActivationFunctionType

B = 8
L = 4096
DIM = 2048
NSLOT = 64
NCH = DIM // 128  # 16 dim chunks
TOK = 512  # tokens per tile
NT = L // TOK  # 8 tiles per core
NQ = TOK // 128  # 4 token quarters per tile


def _build(gate_b: float, interleave: bool = True) -> bass.Bass:
    nc = bacc.Bacc("TRN2", target_bir_lowering=False, debug=False)

    xhd = nc.dram_tensor("xhd", [NT * 128, NCH * TOK], F16, kind="ExternalInput").ap()
    xld = nc.dram_tensor("xld", [NT * 128, NCH * TOK], F16, kind="ExternalInput").ap()
    Ghd = nc.dram_tensor(
        "Ghd", [128, NCH * 128], F16, kind="ExternalInput"
    ).ap()
    Gld = nc.dram_tensor(
        "Gld", [128, NCH * 128], F16, kind="ExternalInput"
    ).ap()
    Wvd = nc.dram_tensor("Wvd", [128, NCH * 128], F16, kind="ExternalInput").ap()
    gvd = nc.dram_tensor("gvd", [1, NQ * NSLOT], F32, kind="ExternalInput").ap()
    outb = nc.dram_tensor(
        "outb", [NT * 128, NCH * TOK], F16, kind="ExternalOutput"
    ).ap()

    # per-tile views; dim d = c*128 + p everywhere
    xh_v = xhd.rearrange("(t p) (c k) -> t p c k", p=128, k=TOK)
    xl_v = xld.rearrange("(t p) (c k) -> t p c k", p=128, k=TOK)
    outb_v = outb.rearrange("(t p) (c k) -> t p c k", p=128, k=TOK)
    Gh_v = Ghd.rearrange("p (c n) -> p c n", n=128)
    Gl_v = Gld.rearrange("p (c n) -> p c n", n=128)
    Wv_v = Wvd.rearrange("n (c j) -> n c j", j=128)

    with tile.TileContext(nc) as tc, ExitStack() as ctx:
        consts = ctx.enter_context(tc.tile_pool(name="consts", bufs=1))
        xpool = ctx.enter_context(tc.tile_pool(name="xpool", bufs=3))
        opool = ctx.enter_context(tc.tile_pool(name="opool", bufs=2))
        work = ctx.enter_context(tc.tile_pool(name="work", bufs=2))
        small = ctx.enter_context(tc.tile_pool(name="small", bufs=2))
        psS = ctx.enter_context(tc.tile_pool(name="psS", bufs=1, space="PSUM"))
        psT = ctx.enter_context(tc.tile_pool(name="psT", bufs=1, space="PSUM"))
        psE = ctx.enter_context(tc.tile_pool(name="psE", bufs=1, space="PSUM"))
        psG = ctx.enter_context(tc.tile_pool(name="psG", bufs=1, space="PSUM"))
        psP = ctx.enter_context(tc.tile_pool(name="psP", bufs=2, space="PSUM"))

        ident = consts.tile([128, 128], F32)
        make_identity(nc, ident)
        ident_16 = consts.tile([128, 128], F16)
        nc.vector.tensor_copy(ident_16, ident)
        Gh_sb = consts.tile([128, NCH, 128], F16)
        nc.sync.dma_start(out=Gh_sb, in_=Gh_v)
        Gl_sb = consts.tile([128, NCH, 128], F16)
        nc.sync.dma_start(out=Gl_sb, in_=Gl_v)
        Wv_sb = consts.tile([128, NCH, 128], F16)
        nc.sync.dma_start(out=Wv_sb, in_=Wv_v)
        # manually double-buffered full-height E tiles; rows 65:128 are
        # zeroed once and never rewritten, so the zero-padded Wv rows
        # always multiply zeros (K=128 keeps the weight load on the
        # fast path)
        E_pair = []
        for i in range(2):
            E_buf = consts.tile([128, NQ, 128], F16, name=f"E_buf{i}")
            nc.vector.memset(E_buf, 0.0)
            E_pair.append(E_buf)
        gv4_flat = consts.tile([128, NQ * NSLOT], F32)
        nc.sync.dma_start(out=gv4_flat, in_=gvd.to_broadcast((128, NQ * NSLOT)))
        gv4 = gv4_flat.rearrange("p (a b) -> p a b", b=NSLOT)
        ones_16 = consts.tile([NSLOT + 1, 128], F16)
        nc.vector.memset(ones_16, 1.0)

        def ph_dma(t):
            xh_sb = xpool.tile([128, NCH, TOK], F16, tag="xh")
            nc.sync.dma_start(out=xh_sb, in_=xh_v[t])
            xl_sb = xpool.tile([128, NCH, TOK], F16, tag="xl")
            nc.sync.dma_start(out=xl_sb, in_=xl_v[t])
            return {"xh": xh_sb, "xl": xl_sb}

        def s_seq(st):
            # fp16-pair split: G@x = Gh@xh + Gl@xh + Gh@xl (+ O(2^-24)),
            # pass-major so the first 32 matmuls only need xh
            seq = []
            for G_c, xkey in ((Gh_sb, "xh"), (Gl_sb, "xh"), (Gh_sb, "xl")):
                for c in range(NCH):
                    seq.append((G_c[:, c, :], st[xkey][:, c, :]))
            return seq

        def ph_S(t, st):
            """S^T = [Ws; gxw] @ x_tile via 3 fp16 passes per chunk."""
            S_ps = psS.tile([128, TOK], F32, tag="S")
            seq = s_seq(st)
            for i, (lhsT, rhs) in enumerate(seq):
                nc.tensor.matmul(
                    S_ps,
                    lhsT,
                    rhs,
                    start=(i == 0),
                    stop=(i == len(seq) - 1),
                )
            st["S_ps"] = S_ps

        def ph_Stok(t, st):
            S_ps = st.pop("S_ps")
            S_sb = work.tile([NSLOT + 1, TOK], F32, tag="S_sb")
            nc.scalar.copy(S_sb, S_ps[0 : NSLOT + 1, :])
            Stok = psT.tile([128, NQ, NSLOT + 1], F32, tag="T")
            for q in range(NQ):
                nc.tensor.transpose(
                    Stok[:, q, :],
                    S_sb[:, ts(q, 128)],
                    ident[0 : NSLOT + 1, 0 : NSLOT + 1],
                )
            st["Stok"] = Stok

        def ph_stats(t, st):
            """Token-major softmax stats; produces Ec (bf16, col 64 = g)."""
            Stok = st.pop("Stok")
            mx4 = small.tile([128, NQ], F32, tag="mx4")
            nc.vector.tensor_reduce(mx4, Stok[:, :, 0:NSLOT], axis=AX_X.X, op=ALU.max)
            mb4 = small.tile([128, NQ], F32, tag="mb4")
            nc.vector.tensor_scalar_mul(mb4, mx4, -10.0)
            Etok = work.tile([128, NQ, NSLOT], F32, tag="Etok")
            s4 = small.tile([128, NQ], F32, tag="s4")
            for q in range(NQ):
                nc.scalar.activation(
                    Etok[:, q, :],
                    Stok[:, q, 0:NSLOT],
                    func=ACTF.Exp,
                    bias=mb4[:, q : q + 1],
                    scale=10.0,
                    accum_out=s4[:, q : q + 1],
                )
            gvd4 = small.tile([128, NQ], F32, tag="gvd4")
            scr = work.tile([128, NQ, NSLOT], F32, tag="scr")
            nc.vector.tensor_mul(scr, Etok, gv4)
            nc.vector.tensor_reduce(gvd4, scr, axis=AX_X.X, op=ALU.add)
            r4 = small.tile([128, NQ], F32, tag="r4")
            nc.vector.reciprocal(r4, s4)
            t4 = small.tile([128, NQ], F32, tag="t4")
            nc.vector.tensor_mul(t4, gvd4, r4)
            gl4 = small.tile([128, NQ], F32, tag="gl4")
            gx_row = Stok[:, :, NSLOT : NSLOT + 1].rearrange("p a b -> p (a b)")
            nc.vector.tensor_add(gl4, t4, gx_row)
            u4 = small.tile([128, NQ], F32, tag="u4")
            nc.scalar.activation(
                u4, gl4, func=ACTF.Exp, bias=-gate_b, scale=-1.0
            )
            den4 = small.tile([128, NQ], F32, tag="den4")
            nc.vector.tensor_scalar_add(den4, u4, 1.0)
            g4 = small.tile([128, NQ], F32, tag="g4")
            nc.vector.reciprocal(g4, den4)
            cp4 = small.tile([128, NQ], F32, tag="cp4")
            nc.vector.tensor_mul(cp4, u4, r4)
            Ec = work.tile([128, NQ, NSLOT + 1], F16, tag="Ec")
            for q in range(NQ):
                nc.vector.tensor_scalar_mul(
                    Ec[:, q, 0:NSLOT], Etok[:, q, :], cp4[:, q : q + 1]
                )
            g_col = Ec[:, :, NSLOT : NSLOT + 1].rearrange("p a b -> p (a b)")
            nc.vector.tensor_copy(g_col, g4)
            st["Ec"] = Ec

        def ph_E(t, st):
            """Ec back to slot-major (f16): E_sb[n, tok], row 64 = g."""
            Ec = st.pop("Ec")
            ET = psE.tile([128, NQ, 128], F16, tag="ET")
            for q in range(NQ):
                nc.tensor.transpose(ET[0 : NSLOT + 1, q, :], Ec[:, q, :], ident_16)
            E_sb = E_pair[t % 2]
            nc.scalar.copy(E_sb[0 : NSLOT + 1, :, :], ET[0 : NSLOT + 1, :, :])
            st["E_sb"] = E_sb

        def ph_combine(t, st, s_next=None):
            """P = x + E''@Wv accumulated on PE per PSUM bank-pair; then
            out = g * P on DVE; one contiguous bf16 DMA per tile."""
            x = st["xh"]
            E_flat = st.pop("E_sb").rearrange("p a b -> p (a b)")  # [128, 512]
            g_ps = psG.tile([128, TOK], F32, tag="G")
            nc.tensor.matmul(
                g_ps,
                ones_16[NSLOT : NSLOT + 1, :],
                E_flat[NSLOT : NSLOT + 1, :],
                start=True,
                stop=True,
            )
            g2 = work.tile([128, 2, TOK], F32, tag="g2")
            nc.scalar.copy(g2[:, 0, :], g_ps)
            nc.scalar.copy(g2[:, 1, :], g_ps)
            out_sb = opool.tile([128, NCH, TOK], F16, tag="o")
            if s_next is not None:
                t2, st2 = s_next
                st2["_Sps"] = psS.tile([128, TOK], F32, tag="S", name="S_ps_il")
                st2["_seq"] = s_seq(st2)

            def s_il(j, i):
                # interleave 3 of next tile's S matmuls between same-bank
                # ident/R pairs so the PE never stalls on a PSUM hazard
                if s_next is None:
                    return
                seq = st2["_seq"]
                for i2 in range(3 * (2 * j + i), 3 * (2 * j + i) + 3):
                    lhsT, rhs = seq[i2]
                    nc.tensor.matmul(
                        st2["_Sps"],
                        lhsT,
                        rhs,
                        start=(i2 == 0),
                        stop=(i2 == len(seq) - 1),
                        skip_group_check=True,
                    )

            for j in range(NCH // 2):
                P = psP.tile([128, 2, TOK], F32, tag="P")
                for k in range(2):
                    nc.tensor.matmul(
                        P[:, k, :],
                        ident_16,
                        x[:, 2 * j + k, :],
                        start=True,
                        stop=False,
                    )
                s_il(j, 0)
                for k in range(2):
                    nc.tensor.matmul(
                        P[:, k, :],
                        Wv_sb[:, 2 * j + k, :],
                        E_flat,
                        start=False,
                        stop=True,
                    )
                s_il(j, 1)
                nc.vector.tensor_mul(out_sb[:, 2 * j : 2 * j + 2, :], P, g2)
            nc.sync.dma_start(out=outb_v[t], in_=out_sb)
            if s_next is not None:
                t2, st2 = s_next
                st2.pop("_seq")
                st2["S_ps"] = st2.pop("_Sps")
                ph_Stok(t2, st2)

        # Software pipeline: per iteration t the PE stream is
        #   ET(t) -> g_bc(t) -> [P pairs(t) + S(t+1) interleaved] -> Stok(t+1)
        # DVE: combine TTs(t) -> stats(t+1);  x DMA runs 2 tiles ahead.
        states = {0: ph_dma(0), 1: ph_dma(1)}
        ph_S(0, states[0])
        ph_Stok(0, states[0])
        ph_stats(0, states[0])
        for t in range(NT):
            if t + 2 < NT:
                states[t + 2] = ph_dma(t + 2)
            ph_E(t, states[t])
            ph_combine(
                t,
                states[t],
                s_next=(t + 1, states[t + 1])
                if (interleave and t + 1 < NT)
                else None,
            )
            if t + 1 < NT:
                if not interleave:
                    ph_S(t + 1, states[t + 1])
                    ph_Stok(t + 1, states[t + 1])
                ph_stats(t + 1, states[t + 1])
            del states[t]

    nc.compile()
    return nc


def _fold_weights(memory, key_w, value_w, gate_w):
    mem = np.asarray(memory, np.float64)
    # query = x @ key_w.T ; scores = query @ memory.T = x @ (memory @ key_w).T
    Ws = (mem @ np.asarray(key_w, np.float64)).astype(np.float32)  # [64, 2048]
    Wv = (mem @ np.asarray(value_w, np.float64).T).astype(np.float32)  # [64, 2048]
    gx = np.asarray(gate_w[0, :DIM], dtype=np.float32)
    gv = (Wv.astype(np.float64) @ np.asarray(gate_w[0, DIM:], np.float64)).astype(
        np.float32
    )
    G = np.concatenate(
        [Ws, gx[None, :], np.zeros((128 - NSLOT - 1, DIM), np.float32)], axis=0
    )  # [128, 2048]: 64 slots, gate row, zero padding (FWL wants 128 cols)
    GT = np.ascontiguousarray(
        G.T.reshape(NCH, 128, 128).transpose(1, 0, 2)
    ).reshape(128, NCH * 128)
    Gh = GT.astype(np.float16)
    Gl = (GT - Gh.astype(np.float32)).astype(np.float16)
    WvT = np.ascontiguousarray(Wv.reshape(NSLOT, NCH * 128))  # [64, 2048]
    gv4 = np.tile(gv, NQ).reshape(1, NQ * NSLOT)
    return Gh, Gl, WvT, gv4


def _tile_x(xb):
    # [L, D] -> [NT*128, NCH*TOK]: blob[t, p, c, k] = x[t*TOK+k, c*128+p]
    return np.ascontiguousarray(
        xb.reshape(NT, TOK, NCH, 128).transpose(0, 3, 2, 1)
    ).reshape(NT * 128, NCH * TOK)


def _untile_out(blob):
    # [NT*128, NCH*TOK] -> [L, D]
    return (
        blob.reshape(NT, 128, NCH, TOK)
        .transpose(0, 3, 2, 1)
        .reshape(L, DIM)
        .astype(np.float32)
    )


def kernel(
    x,
    memory,
    key_w,
    value_w,
    gate_w,
    gate_b,
    _trace=False,
    _tmpdir=None,
):
    x = np.asarray(x, dtype=np.float32)
    Gh, Gl, WvT, gv4 = _fold_weights(
        np.asarray(memory, np.float32),
        np.asarray(key_w, np.float32),
        np.asarray(value_w, np.float32),
        np.asarray(gate_w, np.float32),
    )
    Wv_16 = np.concatenate([WvT, np.zeros_like(WvT)], axis=0).astype(F16_NP)
    nc = _build(float(np.asarray(gate_b).reshape(-1)[0]))
    in_maps = []
    for b in range(B):
        xt = _tile_x(x[b])
        xh = xt.astype(np.float16)
        xl = (xt - xh.astype(np.float32)).astype(np.float16)
        in_maps.append(
            {"xhd": xh, "xld": xl, "Ghd": Gh, "Gld": Gl, "Wvd": Wv_16, "gvd": gv4}
        )
    res = run_bass_kernel_spmd(
        nc, in_maps, list(range(B)), trace=_trace, tmpdir=_tmpdir
    )
    out = np.stack(
        [_untile_out(np.asarray(res.results[b]["outb"])) for b in range(B)], axis=0
    )
    if _trace:
        return out.astype(np.float32), res
    return out.astype(np.float32)


# revision 19
# speedup vs baseline: 1.6953x; 1.0481x over previous
"""MemoryBank kernel for 8x TRN2 NeuronCores (SPMD, batch-parallel).

Algebraic restructure (exact in real arithmetic):
    S        = x @ [Ws; gx_w].T          # [L, 65]; Ws = memory @ key_w (folded)
    E        = exp(10*(S[:, :64] - m))   # m = per-token max
    sum      = E.1,  r = 1/sum
    gl       = (E @ gv) * r + S[:, 64] + b      # gv = Wv @ gate_w[D:]
    u        = exp(-gl); g = 1/(1+u)            # sigmoid without Sigmoid LUT
    out      = g * (x + (E * u * r) @ Wv)       # since (1-g)/g == u exactly

The final form lets the PE accumulate x (identity matmul, f32r) and the
retrieval (bf16) into the SAME PSUM bank, so the elementwise combine is a
single DVE multiply by a broadcast gate row. Scores run f32r (full rate),
retrieval runs bf16, output is written bf16 (host upcasts). Each core
handles one batch element; x is pre-tiled on the host so every DMA is a
single fully-contiguous 2-4 MiB transfer.
"""

from contextlib import ExitStack

import numpy as np

F16_NP = np.float16

import concourse.bass as bass
import concourse.tile as tile
from concourse import bacc
from concourse import mybir
from concourse.bass import ts
from concourse.bass_utils import run_bass_kernel_spmd
from concourse.masks import make_identity

F32 = mybir.dt.float32
F32R = mybir.dt.float32r
F16 = mybir.dt.float16
AX_X = mybir.AxisListType
ALU = mybir.AluOpType
ACTF = mybir.ActivationFunctionType

B = 8
L = 4096
DIM = 2048
NSLOT = 64
NCH = DIM // 128  # 16 dim chunks
TOK = 512  # tokens per tile
NT = L // TOK  # 8 tiles per core
NQ = TOK // 128  # 4 token quarters per tile


def _build(gate_b: float, interleave: bool = True) -> bass.Bass:
    nc = bacc.Bacc("TRN2", target_bir_lowering=False, debug=False)

    xhd = nc.dram_tensor("xhd", [NT * 128, NCH * TOK], F16, kind="ExternalInput").ap()
    xld = nc.dram_tensor("xld", [NT * 128, NCH * TOK], F16, kind="ExternalInput").ap()
    Ghd = nc.dram_tensor(
        "Ghd", [128, NCH * 128], F16, kind="ExternalInput"
    ).ap()
    Gld = nc.dram_tensor(
        "Gld", [128, NCH * 128], F16, kind="ExternalInput"
    ).ap()
    Wvd = nc.dram_tensor("Wvd", [128, NCH * 128], F16, kind="ExternalInput").ap()
    gvd = nc.dram_tensor("gvd", [1, NQ * NSLOT], F32, kind="ExternalInput").ap()
    outb = nc.dram_tensor(
        "outb", [NT * 128, NCH * TOK], F16, kind="ExternalOutput"
    ).ap()

    # per-tile views; dim d = c*128 + p everywhere
    xh_v = xhd.rearrange("(t p) (c k) -> t p c k", p=128, k=TOK)
    xl_v = xld.rearrange("(t p) (c k) -> t p c k", p=128, k=TOK)
    outb_v = outb.rearrange("(t p) (c k) -> t p c k", p=128, k=TOK)
    Gh_v = Ghd.rearrange("p (c n) -> p c n", n=128)
    Gl_v = Gld.rearrange("p (c n) -> p c n", n=128)
    Wv_v = Wvd.rearrange("n (c j) -> n c j", j=128)

    with tile.TileContext(nc) as tc, ExitStack() as ctx:
        consts = ctx.enter_context(tc.tile_pool(name="consts", bufs=1))
        xpool = ctx.enter_context(tc.tile_pool(name="xpool", bufs=3))
        opool = ctx.enter_context(tc.tile_pool(name="opool", bufs=2))
        work = ctx.enter_context(tc.tile_pool(name="work", bufs=2))
        small = ctx.enter_context(tc.tile_pool(name="small", bufs=2))
        psS = ctx.enter_context(tc.tile_pool(name="psS", bufs=1, space="PSUM"))
        psT = ctx.enter_context(tc.tile_pool(name="psT", bufs=1, space="PSUM"))
        psE = ctx.enter_context(tc.tile_pool(name="psE", bufs=1, space="PSUM"))
        psG = ctx.enter_context(tc.tile_pool(name="psG", bufs=1, space="PSUM"))
        psP = ctx.enter_context(tc.tile_pool(name="psP", bufs=2, space="PSUM"))

        ident = consts.tile([128, 128], F32)
        make_identity(nc, ident)
        ident_16 = consts.tile([128, 128], F16)
        nc.vector.tensor_copy(ident_16, ident)
        Gh_sb = consts.tile([128, NCH, 128], F16)
        Gl_sb = consts.tile([128, NCH, 128], F16)
        Wv_sb = consts.tile([128, NCH, 128], F16)
        # manually double-buffered full-height E tiles; rows 65:128 are
        # zeroed once and never rewritten, so the zero-padded Wv rows
        # always multiply zeros (K=128 keeps the weight load on the
        # fast path)
        E_pair = []
        for i in range(2):
            E_buf = consts.tile([128, NQ, 128], F16, name=f"E_buf{i}")
            nc.vector.memset(E_buf, 0.0)
            E_pair.append(E_buf)
        gv4_flat = consts.tile([128, NQ * NSLOT], F32)
        gv4 = gv4_flat.rearrange("p (a b) -> p a b", b=NSLOT)
        ones_16 = consts.tile([NSLOT + 1, 128], F16)
        nc.vector.memset(ones_16, 1.0)

        def ph_dma(t):
            xh_sb = xpool.tile([128, NCH, TOK], F16, tag="xh")
            nc.sync.dma_start(out=xh_sb, in_=xh_v[t])
            xl_sb = xpool.tile([128, NCH, TOK], F16, tag="xl")
            nc.sync.dma_start(out=xl_sb, in_=xl_v[t])
            return {"xh": xh_sb, "xl": xl_sb}

        def s_seq(st):
            # fp16-pair split: G@x = Gh@xh + Gl@xh + Gh@xl (+ O(2^-24)),
            # pass-major so the first 32 matmuls only need xh
            seq = []
            for G_c, xkey in ((Gh_sb, "xh"), (Gl_sb, "xh"), (Gh_sb, "xl")):
                for c in range(NCH):
                    seq.append((G_c[:, c, :], st[xkey][:, c, :]))
            return seq

        def ph_S(t, st):
            """S^T = [Ws; gxw] @ x_tile via 3 fp16 passes per chunk."""
            S_ps = psS.tile([128, TOK], F32, tag="S")
            seq = s_seq(st)
            for i, (lhsT, rhs) in enumerate(seq):
                nc.tensor.matmul(
                    S_ps,
                    lhsT,
                    rhs,
                    start=(i == 0),
                    stop=(i == len(seq) - 1),
                )
            st["S_ps"] = S_ps

        def ph_Stok(t, st):
            S_ps = st.pop("S_ps")
            S_sb = work.tile([NSLOT + 1, TOK], F32, tag="S_sb")
            nc.scalar.copy(S_sb, S_ps[0 : NSLOT + 1, :])
            Stok = psT.tile([128, NQ, NSLOT + 1], F32, tag="T")
            for q in range(NQ):
                nc.tensor.transpose(
                    Stok[:, q, :],
                    S_sb[:, ts(q, 128)],
                    ident[0 : NSLOT + 1, 0 : NSLOT + 1],
                )
            st["Stok"] = Stok

        def ph_stats(t, st):
            """Token-major softmax stats; produces Ec (bf16, col 64 = g)."""
            Stok = st.pop("Stok")
            mx4 = small.tile([128, NQ], F32, tag="mx4")
            nc.vector.tensor_reduce(mx4, Stok[:, :, 0:NSLOT], axis=AX_X.X, op=ALU.max)
            mb4 = small.tile([128, NQ], F32, tag="mb4")
            nc.vector.tensor_scalar_mul(mb4, mx4, -10.0)
            Etok = work.tile([128, NQ, NSLOT], F32, tag="Etok")
            s4 = small.tile([128, NQ], F32, tag="s4")
            for q in range(NQ):
                nc.scalar.activation(
                    Etok[:, q, :],
                    Stok[:, q, 0:NSLOT],
                    func=ACTF.Exp,
                    bias=mb4[:, q : q + 1],
                    scale=10.0,
                    accum_out=s4[:, q : q + 1],
                )
            gvd4 = small.tile([128, NQ], F32, tag="gvd4")
            scr = work.tile([128, NQ, NSLOT], F32, tag="scr")
            nc.vector.tensor_mul(scr, Etok, gv4)
            nc.vector.tensor_reduce(gvd4, scr, axis=AX_X.X, op=ALU.add)
            r4 = small.tile([128, NQ], F32, tag="r4")
            nc.vector.reciprocal(r4, s4)
            t4 = small.tile([128, NQ], F32, tag="t4")
            nc.vector.tensor_mul(t4, gvd4, r4)
            gl4 = small.tile([128, NQ], F32, tag="gl4")
            gx_row = Stok[:, :, NSLOT : NSLOT + 1].rearrange("p a b -> p (a b)")
            nc.vector.tensor_add(gl4, t4, gx_row)
            u4 = small.tile([128, NQ], F32, tag="u4")
            nc.scalar.activation(
                u4, gl4, func=ACTF.Exp, bias=-gate_b, scale=-1.0
            )
            den4 = small.tile([128, NQ], F32, tag="den4")
            nc.vector.tensor_scalar_add(den4, u4, 1.0)
            g4 = small.tile([128, NQ], F32, tag="g4")
            nc.vector.reciprocal(g4, den4)
            cp4 = small.tile([128, NQ], F32, tag="cp4")
            nc.vector.tensor_mul(cp4, u4, r4)
            Ec = work.tile([128, NQ, NSLOT + 1], F16, tag="Ec")
            for q in range(NQ):
                nc.vector.tensor_scalar_mul(
                    Ec[:, q, 0:NSLOT], Etok[:, q, :], cp4[:, q : q + 1]
                )
            g_col = Ec[:, :, NSLOT : NSLOT + 1].rearrange("p a b -> p (a b)")
            nc.vector.tensor_copy(g_col, g4)
            st["Ec"] = Ec

        def ph_E(t, st):
            """Ec back to slot-major (f16): E_sb[n, tok], row 64 = g."""
            Ec = st.pop("Ec")
            ET = psE.tile([128, NQ, 128], F16, tag="ET")
            for q in range(NQ):
                nc.tensor.transpose(ET[0 : NSLOT + 1, q, :], Ec[:, q, :], ident_16)
            E_sb = E_pair[t % 2]
            nc.scalar.copy(E_sb[0 : NSLOT + 1, :, :], ET[0 : NSLOT + 1, :, :])
            st["E_sb"] = E_sb

        def ph_combine(t, st, s_next=None):
            """P = x + E''@Wv accumulated on PE per PSUM bank-pair; then
            out = g * P on DVE; one contiguous bf16 DMA per tile."""
            x = st["xh"]
            E_flat = st.pop("E_sb").rearrange("p a b -> p (a b)")  # [128, 512]
            g_ps = psG.tile([128, TOK], F32, tag="G")
            nc.tensor.matmul(
                g_ps,
                ones_16[NSLOT : NSLOT + 1, :],
                E_flat[NSLOT : NSLOT + 1, :],
                start=True,
                stop=True,
            )
            g2 = work.tile([128, 2, TOK], F32, tag="g2")
            nc.scalar.copy(g2[:, 0, :], g_ps)
            nc.scalar.copy(g2[:, 1, :], g_ps)
            out_sb = opool.tile([128, NCH, TOK], F16, tag="o")
            if s_next is not None:
                t2, st2 = s_next
                st2["_Sps"] = psS.tile([128, TOK], F32, tag="S", name="S_ps_il")
                st2["_seq"] = s_seq(st2)

            def s_il(j, i):
                # interleave next tile's S matmuls between same-bank
                # ident/R pairs so the PE never stalls on a PSUM hazard;
                # front-loaded (4 per half-pair, pairs 0-5) so Stok/stats
                # of the next tile can start before this tile drains
                if s_next is None:
                    return
                seq = st2["_seq"]
                lo = 4 * (2 * j + i)
                for i2 in range(lo, min(lo + 4, len(seq))):
                    lhsT, rhs = seq[i2]
                    nc.tensor.matmul(
                        st2["_Sps"],
                        lhsT,
                        rhs,
                        start=(i2 == 0),
                        stop=(i2 == len(seq) - 1),
                        skip_group_check=True,
                    )
                if lo + 4 == len(seq):
                    st2["S_ps"] = st2.pop("_Sps")
                    ph_Stok(t2, st2)

            for j in range(NCH // 2):
                P = psP.tile([128, 2, TOK], F32, tag="P")
                for k in range(2):
                    nc.tensor.matmul(
                        P[:, k, :],
                        ident_16,
                        x[:, 2 * j + k, :],
                        start=True,
                        stop=False,
                    )
                s_il(j, 0)
                for k in range(2):
                    nc.tensor.matmul(
                        P[:, k, :],
                        Wv_sb[:, 2 * j + k, :],
                        E_flat,
                        start=False,
                        stop=True,
                    )
                s_il(j, 1)
                nc.vector.tensor_mul(out_sb[:, 2 * j : 2 * j + 2, :], P, g2)
                if j == 3:
                    nc.sync.dma_start(
                        out=outb_v[t][:, 0:8, :], in_=out_sb[:, 0:8, :]
                    )
            nc.sync.dma_start(out=outb_v[t][:, 8:16, :], in_=out_sb[:, 8:16, :])
            if s_next is not None:
                st2.pop("_seq")

        # Software pipeline: per iteration t the PE stream is
        #   ET(t) -> g_bc(t) -> [P pairs(t) + S(t+1) interleaved] -> Stok(t+1)
        # DVE: combine TTs(t) -> stats(t+1);  x DMA runs 2 tiles ahead.
        # Prologue DMAs are issued in latency-critical order so S(0) can
        # start after just xh(0) + Gh (~2.5 MiB).
        st0 = {}
        st0["xh"] = xpool.tile([128, NCH, TOK], F16, tag="xh", name="xh0")
        nc.sync.dma_start(out=st0["xh"], in_=xh_v[0])
        nc.sync.dma_start(out=Gh_sb, in_=Gh_v)
        st0["xl"] = xpool.tile([128, NCH, TOK], F16, tag="xl", name="xl0")
        nc.sync.dma_start(out=st0["xl"], in_=xl_v[0])
        nc.sync.dma_start(out=Gl_sb, in_=Gl_v)
        nc.sync.dma_start(out=Wv_sb, in_=Wv_v)
        nc.sync.dma_start(out=gv4_flat, in_=gvd.to_broadcast((128, NQ * NSLOT)))
        states = {0: st0, 1: ph_dma(1)}
        ph_S(0, states[0])
        ph_Stok(0, states[0])
        ph_stats(0, states[0])
        for t in range(NT):
            if t + 2 < NT:
                states[t + 2] = ph_dma(t + 2)
            ph_E(t, states[t])
            ph_combine(
                t,
                states[t],
                s_next=(t + 1, states[t + 1])
                if (interleave and t + 1 < NT)
                else None,
            )
            if t + 1 < NT:
                if not interleave:
                    ph_S(t + 1, states[t + 1])
                    ph_Stok(t + 1, states[t + 1])
                ph_stats(t + 1, states[t + 1])
            del states[t]

    nc.compile()
    return nc


def _fold_weights(memory, key_w, value_w, gate_w):
    mem = np.asarray(memory, np.float64)
    # query = x @ key_w.T ; scores = query @ memory.T = x @ (memory @ key_w).T
    Ws = (mem @ np.asarray(key_w, np.float64)).astype(np.float32)  # [64, 2048]
    Wv = (mem @ np.asarray(value_w, np.float64).T).astype(np.float32)  # [64, 2048]
    gx = np.asarray(gate_w[0, :DIM], dtype=np.float32)
    gv = (Wv.astype(np.float64) @ np.asarray(gate_w[0, DIM:], np.float64)).astype(
        np.float32
    )
    G = np.concatenate(
        [Ws, gx[None, :], np.zeros((128 - NSLOT - 1, DIM), np.float32)], axis=0
    )  # [128, 2048]: 64 slots, gate row, zero padding (FWL wants 128 cols)
    GT = np.ascontiguousarray(
        G.T.reshape(NCH, 128, 128).transpose(1, 0, 2)
    ).reshape(128, NCH * 128)
    Gh = GT.astype(np.float16)
    Gl = (GT - Gh.astype(np.float32)).astype(np.float16)
    WvT = np.ascontiguousarray(Wv.reshape(NSLOT, NCH * 128))  # [64, 2048]
    gv4 = np.tile(gv, NQ).reshape(1, NQ * NSLOT)
    return Gh, Gl, WvT, gv4


def _tile_x(xb):
    # [L, D] -> [NT*128, NCH*TOK]: blob[t, p, c, k] = x[t*TOK+k, c*128+p]
    return np.ascontiguousarray(
        xb.reshape(NT, TOK, NCH, 128).transpose(0, 3, 2, 1)
    ).reshape(NT * 128, NCH * TOK)


def _untile_out(blob):
    # [NT*128, NCH*TOK] -> [L, D]
    return (
        blob.reshape(NT, 128, NCH, TOK)
        .transpose(0, 3, 2, 1)
        .reshape(L, DIM)
        .astype(np.float32)
    )


def kernel(
    x,
    memory,
    key_w,
    value_w,
    gate_w,
    gate_b,
    _trace=False,
    _tmpdir=None,
):
    x = np.asarray(x, dtype=np.float32)
    Gh, Gl, WvT, gv4 = _fold_weights(
        np.asarray(memory, np.float32),
        np.asarray(key_w, np.float32),
        np.asarray(value_w, np.float32),
        np.asarray(gate_w, np.float32),
    )
    Wv_16 = np.concatenate([WvT, np.zeros_like(WvT)], axis=0).astype(F16_NP)
    nc = _build(float(np.asarray(gate_b).reshape(-1)[0]))
    in_maps = []
    for b in range(B):
        xt = _tile_x(x[b])
        xh = xt.astype(np.float16)
        xl = (xt - xh.astype(np.float32)).astype(np.float16)
        in_maps.append(
            {"xhd": xh, "xld": xl, "Ghd": Gh, "Gld": Gl, "Wvd": Wv_16, "gvd": gv4}
        )
    res = run_bass_kernel_spmd(
        nc, in_maps, list(range(B)), trace=_trace, tmpdir=_tmpdir
    )
    out = np.stack(
        [_untile_out(np.asarray(res.results[b]["outb"])) for b in range(B)], axis=0
    )
    if _trace:
        return out.astype(np.float32), res
    return out.astype(np.float32)
